# revision 1
# baseline (speedup 1.0000x reference)
"""Masked multi-head attention (fused QKV) on 8 trn2 NeuronCores.

Problem (full shapes): x [2, 2048, 1024] f32, W [3072, 1024], b [3072].
  z = x @ W.T + b ; k,q,v = split(z) ; heads H=16, hd=64
  out = softmax(causal(q k^T / sqrt(1024))) v   -> [2, 2048, 1024]

Sharding: core c handles batch n=c//4 and head group g=c%4 (4 heads).
Each core is fully independent (data + head parallel, no collectives).
The host pre-transposes x[n] and the per-core W slices; results are
sliced back into out[n, :, 256g:256g+256].

Per-core device program (all matmuls float32r = full-rate PE, f32 bits):
  1) v natural [seq, 4*64] = matmul(lhsT=xT tile, rhs=WvT), bias via a
     K=1 ones-row matmul; stored as [128, ktile, head, 65] with a ones
     column fused in for the softmax denominator.
  2) k,q transposed: zT e-tiles [128, seq] = matmul(lhsT=WkqT tile,
     rhs=xT tile); per-partition bias added on the DVE evacuation. Each
     e-tile holds an even/odd head pair stacked on partitions 0:64/64:128.
  3) Per (q-block 512, head pair): S^T k-tiles [128, 512] via K=64
     matmuls; the even/odd heads issue back-to-back at partition bases
     0/64 so the PE row-packs them concurrently (tile_position
     auto-derived). One ACT exp (scale=1/32, no max subtraction needed)
     evacuates both heads' PSUM banks through a strided [128, 2, 512] AP.
     Causal masking touches only the 4 diagonal k-tiles (gpsimd
     affine_select for the even head, DVE multiply by a device-built
     triangle for the odd head), and fully-masked columns (< 128r on
     diagonal tile r) are trimmed from the matmul, exp, mask, and PV.
  4) outT [65, 512] = [V | 1]^T @ P^T accumulated over k-tiles (row 64 =
     sum of exp). PE-transpose per q-tile, DVE reciprocal of column 64,
     tensor_scalar_mul -> normalized output rows.

Timing (instruction cost model; HW NTFF profiling unavailable through
this axon bridge): ~150 us/core; engine busy PE 110 us, ACT 73 us,
DVE 52 us, gpsimd 19 us, DMA 38 us. The cost model bills the row-packed
score matmul pairs sequentially, so real HW should run ~15 us faster.
Verified on hardware: scale-relative absmax 1.8e-4 vs the fp32 reference.

_split_matmul_waits() is a required legalization for this compiler
build: every engine instruction may carry at most one semaphore wait.
"""

import numpy as np

import concourse.bass as bass
import concourse.mybir as mybir
import concourse.tile as tile
from concourse.bass_utils import run_bass_kernel_spmd
from concourse.masks import make_identity

F32 = mybir.dt.float32
F32R = mybir.dt.float32r  # matmul compute dtype (4-byte, np.float32 on host)

N, S, D = 2, 2048, 1024
H, HD = 16, 64
P = 128
QB = 512                 # q block (free dim per matmul)
NQB = S // QB            # 4
NKT = S // P             # 16 k tiles
ND = D // P              # 8 contraction tiles
NHC = 4                  # heads per core
EKQ = 2 * NHC * HD       # 512 = k+q rows per core
EV = NHC * HD            # 256 = v rows per core
SCALE = 1.0 / 32.0       # 1/sqrt(1024)

AF = mybir.ActivationFunctionType
ALU = mybir.AluOpType


def _split_matmul_waits(nc):
    """Move semaphore waits off Matmult instructions onto preceding PE NOPs.

    The walrus codegen for self-loading fp32/fp32r matmuls folds waits into
    the LDWEIGHTS struct, which has room for a single sync-wait command;
    two producers (e.g. two DMA queues) make it fail with "Too many sync
    wait commands". Sequencer NOPs on the same engine execute in program
    order, so hoisting each wait onto its own NOP is semantics-preserving.
    """
    import bass_rust

    moved = 0
    for bb in nc.main_func.blocks:
        out = []
        for ins in bb.instructions:
            si = ins.sync_info
            keep = 0 if isinstance(ins, bass_rust.InstMatmult) else 1
            if (
                not isinstance(ins, bass_rust.InstNoOp)
                and si is not None
                and len(si.on_wait) > keep
            ):
                hoist = si.on_wait[keep:] if keep else si.on_wait
                for j, w in enumerate(hoist):
                    out.append(
                        bass_rust.InstNoOp(
                            name=f"{ins.name}-hw{j}",
                            engine=ins.engine,
                            sync_info=mybir.SyncInfo(on_wait=[w], on_update=[]),
                        )
                    )
                    moved += 1
                ins.sync_info = mybir.SyncInfo(
                    on_wait=list(si.on_wait[:keep]), on_update=list(si.on_update)
                )
            out.append(ins)
        bb.instructions[:] = out
    return moved


def build_nc():
    nc = bass.Bass()

    xT = nc.dram_tensor("xT", [D, S], F32R, kind="ExternalInput")
    wkq = nc.dram_tensor("wkq", [D, EKQ], F32R, kind="ExternalInput")
    wv = nc.dram_tensor("wv", [D, EV], F32R, kind="ExternalInput")
    bkq = nc.dram_tensor("bkq", [P, 4], F32, kind="ExternalInput")
    bv = nc.dram_tensor("bv", [1, EV], F32R, kind="ExternalInput")
    o = nc.dram_tensor("o", [S, EV], F32, kind="ExternalOutput")

    xT_v = xT.rearrange("(dt p) s -> p dt s", p=P)       # [128, 8, 2048]
    o_v = o.rearrange("(qt p) c -> p qt c", p=P)         # [128, 16, 256]

    with tile.TileContext(nc) as tc:
        with (
            tc.tile_pool(name="const", bufs=1) as const,
            tc.tile_pool(name="big", bufs=1) as big,
            tc.tile_pool(name="xpool", bufs=2) as xpool,
            tc.tile_pool(name="work", bufs=2) as work,
            tc.tile_pool(name="opool", bufs=2) as opool,
            tc.tile_pool(name="proj_ps", bufs=2, space="PSUM") as proj_ps,
            tc.tile_pool(name="st_ps", bufs=2, space="PSUM") as st_ps,
            tc.tile_pool(name="pv_ps", bufs=2, space="PSUM") as pv_ps,
        ):
            # ---- constants ----
            ident = const.tile([P, P], F32)
            make_identity(nc, ident)
            onef = const.tile([P, 1], F32)
            nc.vector.memset(onef, 1.0)
            # warm the ACT exp table while DMAs run
            dummy = const.tile([1, 2], F32)
            nc.gpsimd.memset(dummy, 0.0)
            nc.scalar.activation(dummy, dummy, AF.Exp)

            # interleave the qb0-critical stream: bias, then per-d-tile
            # (wv, xT) chunk pairs so the first projection matmuls start
            # ~1us in; the kq weights follow while v-projection runs.
            bvb = const.tile([P, EV], F32R)
            nc.sync.dma_start(bvb, bv[:, :].partition_broadcast(P))
            wv_sb = const.tile([P, ND, EV], F32R)
            wv_v = wv.rearrange("(dt p) e -> p dt e", p=P)
            xqb0 = xpool.tile([P, ND, QB], F32R, tag="xqb")
            for dt in range(ND):
                nc.sync.dma_start(wv_sb[:, dt], wv_v[:, dt])
                nc.sync.dma_start(xqb0[:, dt], xT_v[:, dt, 0:QB])
            wkq_sb = const.tile([P, ND, EKQ], F32R)
            wkq_v = wkq.rearrange("(dt p) e -> p dt e", p=P)
            for dt in range(ND):
                nc.sync.dma_start(wkq_sb[:, dt], wkq_v[:, dt])
            bkq_sb = const.tile([P, 4], F32)
            nc.sync.dma_start(bkq_sb, bkq[:, :])

            # ---- persistent state ----
            # zT for k,q: e-tiles 0,1 = [k_h0;k_h1],[k_h2;k_h3]; 2,3 = q same
            zkq = big.tile([P, 4, S], F32R)
            # v natural + ones column: [p, ktile, head, 65]
            vsb = big.tile([P, NKT, NHC, HD + 1], F32R)
            nc.vector.tensor_copy(
                vsb[:, :, :, HD:HD + 1],
                onef[:, :, None].to_broadcast((P, NKT, NHC, 1)),
            )  # ones column for the fused sum(exp) row
            # diagonal causal masks: mask[p, r, q] = 1 if q >= p + 128r
            mask_sb = const.tile([P, 1, QB], F32R)
            nc.gpsimd.affine_select(
                out=mask_sb[:, 0, :],
                in_=onef.to_broadcast((P, QB)).bitcast(F32R),
                compare_op=ALU.is_ge, fill=0.0,
                base=0, channel_multiplier=-1,
                pattern=[[1, QB]],
            )
            # exp(S^T) for current (q-block, head pair): [p, head, ktile, q]
            pt = big.tile([P, 2, NKT, QB], F32R)

            def proj_v(qb, xqb):
                # ---- projection: v natural for 4 q-tiles ----
                for qt4 in range(4):
                    qt = qb * 4 + qt4
                    pvp = proj_ps.tile([P, QB], F32, tag="projps")
                    for dt in range(ND):
                        nc.tensor.matmul(
                            pvp[:, :EV],
                            lhsT=(xqb[:, dt, qt4 * P:(qt4 + 1) * P]),
                            rhs=(wv_sb[:, dt, :]),
                            start=(dt == 0), stop=(dt == ND - 1),
                        )
                    nc.vector.tensor_tensor(
                        vsb[:, qt, :, 0:HD],
                        pvp[:, :EV].rearrange("p (h d) -> p h d", d=HD),
                        bvb.rearrange("p (h d) -> p h d", d=HD),
                        mybir.AluOpType.add,
                    )

            def proj_kq(qb, xqb, tiles):
                # ---- projection: zT for k,q e-tiles ----
                for t in tiles:
                    pzp = proj_ps.tile([P, QB], F32, tag="projps")
                    for dt in range(ND):
                        nc.tensor.matmul(
                            pzp,
                            lhsT=(wkq_sb[:, dt, t * P:(t + 1) * P]),
                            rhs=(xqb[:, dt, :]),
                            start=(dt == 0), stop=(dt == ND - 1),
                        )
                    nc.vector.tensor_scalar_add(
                        zkq[:, t, qb * QB:(qb + 1) * QB], pzp, bkq_sb[:, t:t + 1]
                    )


            def attn_st_phase(qb, hp):
                # ---- attention for this q block, per head PAIR ----
                # Heads 2hp (rows 0:64 of e-tiles) and 2hp+1 (rows 64:128)
                # run as row-tiled K=64 matmuls packed into the PE array
                # concurrently (tile_position auto-derived from partition
                # base), one PSUM bank each; exp covers both via a strided
                # [p, 2, 512] AP into pt.
                nkt = 4 * (qb + 1)
                kt_order = list(range(4 * qb, nkt)) + list(range(4 * qb))
                if True:
                    kt_t = hp
                    qt_t = 2 + hp
                    for kt in kt_order:
                        # diagonal tiles: columns < 128r are fully masked,
                        # trim them from the matmul, exp, mask and PV
                        r = kt - 4 * qb
                        off = P * r if 0 <= r < 4 else 0
                        w = QB - off
                        stp = st_ps.tile([P, 2 * QB], F32, tag="st")
                        for hl in range(2):
                            base = HD * hl
                            nc.tensor.matmul(
                                stp[:, hl * QB:hl * QB + w],
                                lhsT=zkq[base:base + HD, kt_t,
                                         kt * P:(kt + 1) * P],
                                rhs=zkq[base:base + HD, qt_t,
                                        qb * QB + off:(qb + 1) * QB],
                                start=True, stop=True,
                            )
                        nc.scalar.activation(
                            pt[:, :, kt, off:QB],
                            stp.rearrange("p (h s) -> p h s", s=QB)[:, :, 0:w],
                            AF.Exp, scale=SCALE,
                        )
                        if 0 <= r < 4:
                            nc.gpsimd.affine_select(
                                out=pt[:, 0, kt, off:QB],
                                in_=pt[:, 0, kt, off:QB],
                                compare_op=ALU.is_ge, fill=0.0,
                                base=0, channel_multiplier=-1,
                                pattern=[[1, w]],
                            )
                            nc.vector.tensor_mul(
                                out=pt[:, 1, kt, off:QB],
                                in0=pt[:, 1, kt, off:QB],
                                in1=mask_sb[:, 0, 0:w],
                            )


            def attn_pv_phase(qb, hp, osb):
                nkt = 4 * (qb + 1)
                kt_order = list(range(4 * qb, nkt)) + list(range(4 * qb))
                if True:
                    # both heads' accumulators live in the 2 PSUM slots so
                    # the PV matmuls interleave per k-tile; the unit then
                    # ends ~one matmul after the last exp instead of a full
                    # second exp-paced pass
                    pvo0 = pv_ps.tile([HD + 1, QB], F32, tag="pv")
                    pvo1 = pv_ps.tile([HD + 1, QB], F32, tag="pv")
                    pvos = [pvo0, pvo1]
                    for i, kt in enumerate(kt_order):
                        r = kt - 4 * qb
                        off = P * r if 0 <= r < 4 else 0
                        for hl in range(2):
                            nc.tensor.matmul(
                                pvos[hl][:, off:QB],
                                lhsT=(vsb[:, kt, 2 * hp + hl, :]),
                                rhs=(pt[:, hl, kt, off:QB]),
                                start=(i == 0), stop=(i == nkt - 1),
                            )
                    for hl in range(2):
                        h = 2 * hp + hl
                        ot = work.tile([HD + 1, QB], F32, tag="ot")
                        nc.vector.tensor_copy(ot, pvos[hl])

                        # transpose + normalize per q-tile of 128
                        for qt4 in range(4):
                            trp = pv_ps.tile([P, HD + 1], F32, tag="pv")
                            nc.tensor.transpose(
                                trp, ot[:, qt4 * P:(qt4 + 1) * P],
                                ident[:HD + 1, :HD + 1],
                            )
                            rs = work.tile([P, 1], F32, tag="rs")
                            nc.vector.reciprocal(rs, trp[:, HD:HD + 1])
                            nc.vector.tensor_scalar_mul(
                                osb[:, qt4, HD * h:HD * (h + 1)],
                                trp[:, 0:HD], rs,
                            )


            # Within each q block, start the first head pair's score
            # matmuls as soon as its two kq e-tiles exist: the serial ACT
            # exp chain (the pacer of the late blocks) begins ~5us earlier.
            for qb in range(NQB):
                if qb == 0:
                    xqb = xqb0
                else:
                    xqb = xpool.tile([P, ND, QB], F32R, tag="xqb")
                    for dt in range(ND):
                        nc.sync.dma_start(
                            xqb[:, dt], xT_v[:, dt, qb * QB:(qb + 1) * QB]
                        )
                osb = opool.tile([P, 4, EV], F32, tag="osb")
                proj_kq(qb, xqb, (0, 2))
                attn_st_phase(qb, 0)
                proj_kq(qb, xqb, (1, 3))
                proj_v(qb, xqb)
                attn_pv_phase(qb, 0, osb)
                nc.sync.dma_start(
                    o_v[:, qb * 4:(qb + 1) * 4, 0:P], osb[:, :, 0:P]
                )
                attn_st_phase(qb, 1)
                attn_pv_phase(qb, 1, osb)
                nc.sync.dma_start(
                    o_v[:, qb * 4:(qb + 1) * 4, P:2 * P], osb[:, :, P:2 * P]
                )
    _split_matmul_waits(nc)
    return nc


_nc_cache = None


def _get_nc():
    global _nc_cache
    if _nc_cache is None:
        _nc_cache = build_nc()
    return _nc_cache


def make_in_maps(x, W, b):
    x = np.asarray(x, dtype=np.float32)
    W = np.asarray(W, dtype=np.float32)
    b = np.asarray(b, dtype=np.float32)
    in_maps = []
    xTs = [np.ascontiguousarray(x[n].T) for n in range(N)]
    for c in range(8):
        n, g = divmod(c, 4)
        rk = slice(256 * g, 256 * g + 256)
        rq = slice(D + 256 * g, D + 256 * g + 256)
        rv = slice(2 * D + 256 * g, 2 * D + 256 * g + 256)
        wkq = np.ascontiguousarray(np.concatenate([W[rk], W[rq]], axis=0).T)
        wv = np.ascontiguousarray(W[rv].T)
        bkq = np.ascontiguousarray(
            np.concatenate([b[rk], b[rq]]).reshape(4, P).T
        )
        bv = np.ascontiguousarray(b[rv].reshape(1, EV))
        in_maps.append(
            {"xT": xTs[n], "wkq": wkq, "wv": wv, "bkq": bkq, "bv": bv}
        )
    return in_maps


def run(inputs, **kwargs):
    nc = _get_nc()
    in_maps = make_in_maps(inputs["x"], inputs["W"], inputs["b"])
    res = run_bass_kernel_spmd(nc, in_maps, core_ids=list(range(8)), **kwargs)
    out = np.empty((N, S, D), dtype=np.float32)
    for c in range(8):
        n, g = divmod(c, 4)
        out[n, :, 256 * g:256 * g + 256] = res.results[c]["o"]
    return out, res


def kernel(**inputs):
    out, _ = run(inputs)
    return out



# revision 23
# speedup vs baseline: 1.1127x; 1.1127x over previous
"""Masked multi-head attention (fused QKV) on 8 trn2 NeuronCores.

Problem (full shapes): x [2, 2048, 1024] f32, W [3072, 1024], b [3072].
  z = x @ W.T + b ; k,q,v = split(z) ; heads H=16, hd=64
  out = softmax(causal(q k^T / sqrt(1024))) v   -> [2, 2048, 1024]

Sharding: core c handles batch n=c//4 and head group g=c%4 (4 heads).
Each core is fully independent (data + head parallel, no collectives).
The host pre-transposes x[n] and the per-core W slices (cast to fp16);
results are sliced back into out[n, :, 256g:256g+256].

Per-core device program (all matmuls fp16 in / f32 PSUM accumulate):
  1) v natural [seq, 4*64] = matmul(lhsT=xT tile, rhs=WvT); bias added on
     the DVE evacuation; stored fp16 as [128, ktile, head, 65] with a
     ones column fused in for the softmax denominator.
  2) k,q transposed: zT e-tiles [128, seq] = matmul(lhsT=WkqT tile,
     rhs=xT tile); per-partition bias added on the DVE evacuation (fp16
     out). Each e-tile holds an even/odd head pair on partitions
     0:64/64:128.
  3) Per (q-block 512, head pair): S^T k-tiles [128, w<=512] via K=64
     matmuls at partition bases 0/64. One ACT exp (scale=1/32, no max
     subtraction needed) per k-tile evacuates both heads' PSUM banks
     through a strided [128, 2, w] AP into fp16 pt. Causal masking
     touches only the 4 diagonal k-tiles (DVE multiply by a triangle
     mask, both heads in one instruction), and fully-masked columns
     (< 128r on diagonal tile r) are trimmed everywhere.
  4) PV flipped: out[q, d] accumulated in PSUM as [128 q, 65] tiles with
     lhsT = S^T slice [128 kpos, 128 q] (stationary; ldweights is free)
     and rhs = [V | 1] [128 kpos, 65] (moving, only 65 columns billed).
     Column 64 = sum of exp. Output lands in natural [seq, e] layout --
     no transposes. DVE reciprocal of column 64 + broadcast multiply
     normalize straight into the output staging tile.

Scheduling: the serial ACT exp chain (~1 us per k-tile) paces the
attention stream while each score matmul pair costs the PE only ~0.4 us,
and the PE issues strictly in order. The emitter therefore interleaves
"filler" PE work -- projection d-tile steps for the next q-block and PV
accumulation chunks once their exponentials have landed -- between score
tiles, budgeted per gap, so neither PE nor ACT ever waits on the other.

_split_matmul_waits() is a required legalization for this compiler
build: every engine instruction may carry at most one semaphore wait.
"""

from collections import deque

import numpy as np

import concourse.bass as bass
import concourse.mybir as mybir
import concourse.tile as tile
from concourse.bass_utils import run_bass_kernel_spmd

F32 = mybir.dt.float32
F16 = mybir.dt.float16

N, S, D = 2, 2048, 1024
H, HD = 16, 64
P = 128
QB = 512                 # q block (free dim per matmul)
NQB = S // QB            # 4
NKT = S // P             # 16 k tiles
ND = D // P              # 8 contraction tiles
NHC = 4                  # heads per core
EKQ = 2 * NHC * HD       # 512 = k+q rows per core
EV = NHC * HD            # 256 = v rows per core
SCALE = 1.0 / 32.0       # 1/sqrt(1024)
PE_NS = 0.4167           # ns per PE row at full p-state

AF = mybir.ActivationFunctionType
ALU = mybir.AluOpType


def _split_matmul_waits(nc):
    """Move semaphore waits off Matmult instructions onto preceding PE NOPs.

    The walrus codegen for self-loading fp32/fp32r matmuls folds waits into
    the LDWEIGHTS struct, which has room for a single sync-wait command;
    two producers (e.g. two DMA queues) make it fail with "Too many sync
    wait commands". Sequencer NOPs on the same engine execute in program
    order, so hoisting each wait onto its own NOP is semantics-preserving.
    """
    import bass_rust

    moved = 0
    for bb in nc.main_func.blocks:
        out = []
        for ins in bb.instructions:
            si = ins.sync_info
            keep = 0 if isinstance(ins, bass_rust.InstMatmult) else 1
            if (
                not isinstance(ins, bass_rust.InstNoOp)
                and si is not None
                and len(si.on_wait) > keep
            ):
                hoist = si.on_wait[keep:] if keep else si.on_wait
                for j, w in enumerate(hoist):
                    out.append(
                        bass_rust.InstNoOp(
                            name=f"{ins.name}-hw{j}",
                            engine=ins.engine,
                            sync_info=mybir.SyncInfo(on_wait=[w], on_update=[]),
                        )
                    )
                    moved += 1
                ins.sync_info = mybir.SyncInfo(
                    on_wait=list(si.on_wait[:keep]), on_update=list(si.on_update)
                )
            out.append(ins)
        bb.instructions[:] = out
    return moved


def build_nc(split_waits=True):
    nc = bass.Bass()

    xT = nc.dram_tensor("xT", [D, S], F16, kind="ExternalInput")
    wkq = nc.dram_tensor("wkq", [D, EKQ], F16, kind="ExternalInput")
    wv = nc.dram_tensor("wv", [D, EV], F16, kind="ExternalInput")
    bkq = nc.dram_tensor("bkq", [P, 4], F32, kind="ExternalInput")
    bv = nc.dram_tensor("bv", [1, EV], F32, kind="ExternalInput")
    o = nc.dram_tensor("o", [S, EV], F32, kind="ExternalOutput")

    xT_v = xT.rearrange("(dt p) s -> p dt s", p=P)       # [128, 8, 2048]
    o_v = o.rearrange("(qt p) c -> p qt c", p=P)         # [128, 16, 256]

    with tile.TileContext(nc) as tc:
        with (
            tc.tile_pool(name="const", bufs=1) as const,
            tc.tile_pool(name="big", bufs=1) as big,
            tc.tile_pool(name="xpool", bufs=4) as xpool,
            tc.tile_pool(name="work", bufs=2) as work,
            tc.tile_pool(name="opool", bufs=4) as opool,
            tc.tile_pool(name="mm_ps", bufs=2, space="PSUM") as mm_ps,
            tc.tile_pool(name="st_ps", bufs=3, space="PSUM") as st_ps,
        ):
            # ---- input DMAs. Host lays wkq columns out as
            # [k01 | q01 | k23 | q23] so head pair 0's weights (one 512B-
            # contiguous chunk) can land first; x streams in 2-dtile chunks
            # matching the projection item granularity, so the first
            # matmuls start ~3.5us in instead of waiting ~9us for
            # everything.
            # Issue overheads spread over three HWDGE queues (SP, ACT,
            # DVE are all idle at start); transfers serialize through the
            # shared DMA engines in issue order, so the big non-critical
            # loads (x1..x3, wv) queue on SP *behind* the startup set.
            xqbs = [None] * NQB
            wkq_v = wkq.rearrange("(dt p) e -> p dt e", p=P)
            wkq_sb = const.tile([P, ND, EKQ], F16)
            bkq_sb = const.tile([P, 4], F32)
            wv_sb = const.tile([P, ND, EV], F16)
            bvb = const.tile([P, EV], F32)
            for i in range(NQB):
                xqbs[i] = xpool.tile(
                    [P, ND, QB], F16, tag="xqb", name=f"xqb{i}"
                )
            nc.sync.dma_start(wkq_sb[:, :, 0:EKQ // 2], wkq_v[:, :, 0:EKQ // 2])
            nc.scalar.dma_start(
                wkq_sb[:, :, EKQ // 2:], wkq_v[:, :, EKQ // 2:]
            )
            nc.scalar.dma_start(bkq_sb, bkq[:, :])
            nc.scalar.dma_start(bvb, bv[:, :].partition_broadcast(P))
            for dc in range(0, ND, 2):
                nc.sync.dma_start(
                    xqbs[0][:, dc:dc + 2], xT_v[:, dc:dc + 2, 0:QB]
                )
            nc.sync.dma_start(xqbs[1], xT_v[:, :, QB:2 * QB])
            nc.sync.dma_start(wv_sb, wv.rearrange("(dt p) e -> p dt e", p=P))
            nc.sync.dma_start(xqbs[2], xT_v[:, :, 2 * QB:3 * QB])
            nc.sync.dma_start(xqbs[3], xT_v[:, :, 3 * QB:4 * QB])

            # ---- constants ----
            onef = const.tile([P, 1], F32)
            nc.vector.memset(onef, 1.0)
            # warm the ACT exp table while DMAs run
            dummy = const.tile([1, 2], F32)
            nc.gpsimd.memset(dummy, 0.0)
            nc.scalar.activation(dummy, dummy, AF.Exp)

            # ---- persistent state ----
            # zT for k,q: e-tiles 0,1 = [k_h0;k_h1],[k_h2;k_h3]; 2,3 = q same
            zkq = big.tile([P, 4, S], F16)
            # v natural + ones column: [p, ktile, head, 65]
            vsb = big.tile([P, NKT, NHC, HD + 1], F16)
            nc.vector.tensor_copy(
                vsb[:, :, :, HD:HD + 1],
                onef[:, :, None].to_broadcast((P, NKT, NHC, 1)),
            )  # ones column for the fused sum(exp) row
            # diagonal causal mask: mask[p, q] = 1 if q >= p (same for every
            # diagonal tile after its dead columns are trimmed)
            mask32 = const.tile([P, QB], F32)
            nc.gpsimd.affine_select(
                out=mask32,
                in_=onef.to_broadcast((P, QB)),
                compare_op=ALU.is_ge, fill=0.0,
                base=0, channel_multiplier=-1,
                pattern=[[1, QB]],
            )
            mask16 = const.tile([P, QB], F16)
            nc.vector.tensor_copy(mask16, mask32)
            # exp(S^T): hp0's plane is double-buffered by q-block parity so
            # the PV of block qb can keep draining while block qb+1's SC0
            # exps land; hp1's PV always drains before the next SC1 begins.
            # layout: pt0 [p, parity, head, ktile, q], pt1 [p, head, ktile, q]
            pt0 = big.tile([P, 2, 2, NKT, QB], F16)
            pt1 = big.tile([P, 2, NKT, QB], F16)

            # ---- filler queue: (est_pe_ns, emit_fn) ----
            fill = deque()

            def run_fill(budget):
                while fill and budget > 0:
                    cost, fn = fill.popleft()
                    fn()
                    budget -= cost

            def drain_fill():
                run_fill(float("inf"))

            ESLOT = {0: 0, 2: 1, 1: 2, 3: 3}  # e-tile -> host column block

            def proj_kq_items(qb, t, xqb):
                # zT e-tile t for q-block qb, split into 2-dtile steps
                state = {}
                s = ESLOT[t]

                def step(d0, first, last):
                    def fn():
                        if first:
                            state["ps"] = mm_ps.tile(
                                [P, QB], F32, tag="mmps", name="kqps"
                            )
                        ps = state["ps"]
                        for dt in range(d0, d0 + 2):
                            nc.tensor.matmul(
                                ps,
                                lhsT=(wkq_sb[:, dt, s * P:(s + 1) * P]),
                                rhs=(xqb[:, dt, :]),
                                start=(dt == 0), stop=(dt == ND - 1),
                            )
                        if last:
                            nc.vector.tensor_scalar_add(
                                zkq[:, t, qb * QB:(qb + 1) * QB],
                                ps, bkq_sb[:, s:s + 1],
                            )
                    return fn

                return [
                    (2 * QB * PE_NS, step(d0, d0 == 0, d0 == ND - 2))
                    for d0 in range(0, ND, 2)
                ]

            def proj_v_items(qb, xqb):
                # v natural for the 4 q-tiles of qb, 4-dtile steps
                items = []
                for qt4 in range(4):
                    qt = qb * 4 + qt4
                    state = {}

                    def step(d0, first, last, qt=qt, qt4=qt4, state=state):
                        def fn():
                            if first:
                                state["ps"] = mm_ps.tile(
                                    [P, QB], F32, tag="mmps", name="vps"
                                )
                            ps = state["ps"]
                            for dt in range(d0, d0 + 4):
                                nc.tensor.matmul(
                                    ps[:, :EV],
                                    lhsT=(xqb[:, dt, qt4 * P:(qt4 + 1) * P]),
                                    rhs=(wv_sb[:, dt, :]),
                                    start=(dt == 0), stop=(dt == ND - 1),
                                )
                            if last:
                                nc.vector.tensor_tensor(
                                    vsb[:, qt, :, 0:HD],
                                    ps[:, :EV].rearrange(
                                        "p (h d) -> p h d", d=HD
                                    ),
                                    bvb.rearrange("p (h d) -> p h d", d=HD),
                                    mybir.AluOpType.add,
                                )
                        return fn

                    for d0 in range(0, ND, 4):
                        items.append(
                            (4 * EV * PE_NS, step(d0, d0 == 0, d0 == ND - 4))
                        )
                return items

            def pv_items(qb, hp, osb):
                # flipped PV + normalize for head pair hp of q-block qb;
                # each chunk carries the max k-tile whose exp it needs so
                # the emitter can flow it into the score loop as soon as
                # that exp has retired.
                items = []
                for qt4 in range(4):
                    nkt = 4 * qb + qt4 + 1
                    mms = [(kt, hl) for kt in range(nkt) for hl in range(2)]
                    state = {}
                    CH = 8
                    chunks = [mms[i:i + CH] for i in range(0, len(mms), CH)]
                    for ci, chunk in enumerate(chunks):
                        first = ci == 0
                        last = ci == len(chunks) - 1

                        def fn(chunk=chunk, first=first, last=last,
                               qt4=qt4, nkt=nkt, state=state):
                            if first:
                                state["pvt"] = mm_ps.tile(
                                    [P, 2, HD + 1], F32, tag="mmps",
                                    name="pvt"
                                )
                            pvt = state["pvt"]
                            src_pt = pt0[:, qb % 2] if hp == 0 else pt1
                            for kt, hl in chunk:
                                nc.tensor.matmul(
                                    pvt[:, hl, :],
                                    lhsT=src_pt[:, hl, kt,
                                                qt4 * P:(qt4 + 1) * P],
                                    rhs=vsb[:, kt, 2 * hp + hl, :],
                                    start=(kt == 0 and hl == 0),
                                    stop=(kt == nkt - 1 and hl == 1),
                                    skip_group_check=True,
                                )
                            if last:
                                rs = work.tile([P, 2], F32, tag="rs")
                                nc.vector.reciprocal(rs, pvt[:, :, HD])
                                nc.vector.tensor_tensor(
                                    osb[:, qt4, 2 * hp * HD:(2 * hp + 2) * HD]
                                    .rearrange("p (h d) -> p h d", d=HD),
                                    pvt[:, :, 0:HD],
                                    rs[:, :, None].to_broadcast((P, 2, HD)),
                                    mybir.AluOpType.mult,
                                )
                                if qb == NQB - 1 and hp == 1:
                                    nc.sync.dma_start(
                                        o_v[:, qb * 4 + qt4, :],
                                        osb[:, qt4, :],
                                    )
                        maxkt = max(kt for kt, _ in chunk)
                        items.append(
                            (len(chunk) * (HD + 1) * PE_NS, fn, maxkt,
                             first, last)
                        )
                return items

            def sc(qb, hp, kt):
                # score matmul pair + exp (+ diagonal mask); returns the
                # filler budget this k-tile's exp buys on the PE.
                r = kt - 4 * qb
                off = P * r if 0 <= r < 4 else 0
                w = QB - off
                stp = st_ps.tile([P, 2, QB], F32, tag="st")
                for hl in range(2):
                    base = HD * hl
                    nc.tensor.matmul(
                        stp[:, hl, 0:w],
                        lhsT=zkq[base:base + HD, hp, kt * P:(kt + 1) * P],
                        rhs=zkq[base:base + HD, 2 + hp,
                                qb * QB + off:(qb + 1) * QB],
                        start=True, stop=True,
                    )
                dst = pt0[:, qb % 2] if hp == 0 else pt1
                nc.scalar.activation(
                    dst[:, :, kt, off:QB],
                    stp[:, :, 0:w],
                    AF.Exp, scale=SCALE,
                )
                if 0 <= r < 4:
                    nc.vector.tensor_tensor(
                        out=dst[:, :, kt, off:QB],
                        in0=dst[:, :, kt, off:QB],
                        in1=mask16[:, None, 0:w].to_broadcast((P, 2, w)),
                        op=mybir.AluOpType.mult,
                    )
                # exp busy (0.833/row + overhead) minus this pair's PE time
                return 2 * w * (0.8333 - PE_NS) + 185.0

            # ---- model-driven emission ----
            # A small list scheduler with virtual PE/ACT clocks decides, at
            # every step, whether to emit the next backbone score tile
            # (keeping the serial ACT exp chain fed) or a filler item:
            # projection chain steps (deadline order, gated on a DMA-arrival
            # model, at most two chains in flight for the 2 PSUM bufs) or PV
            # chunks gated on modeled exp completion. This keeps PE and ACT
            # dense through every q-block boundary without hand-tuned drains.
            osbs = [None] * NQB
            for qb in range(NQB):
                osbs[qb] = opool.tile(
                    [P, 4, EV], F32, tag="osb", name=f"osb{qb}"
                )

            EXP_NS = 0.8333
            EXP_OH = 185.0
            MASK_NS = 300.0
            BACKLOG = 1500.0

            backbone = []  # (qb, hp, kt, mm_ns, exp_ns, diag)
            for qb in range(NQB):
                for hp in range(2):
                    for kt in range(4 * (qb + 1)):
                        r = kt - 4 * qb
                        w = QB - (P * r if 0 <= r < 4 else 0)
                        backbone.append(
                            (qb, hp, kt, 2 * w * PE_NS,
                             2 * w * EXP_NS + EXP_OH, 0 <= r < 4)
                        )
            NSC = len(backbone)
            sc_start = {}
            gidx = 0
            for qb in range(NQB):
                for hp in range(2):
                    sc_start[(qb, hp)] = gidx
                    gidx += 4 * (qb + 1)

            # DMA arrival model (ns): merged transfer order across the
            # three issue queues above, serialized through the DMA engines.
            arr = {
                "wkqA": 3200.0, "wkqB": 4700.0, "bkq": 4750.0,
                "bv": 4800.0, "x0c0": 5500.0, "x0c1": 6300.0,
                "x0c2": 7000.0, "x0c3": 7700.0, "x1": 10600.0,
                "wv": 12100.0, "x2": 15000.0, "x3": 17900.0,
            }

            def xavail(qb, d0):
                if qb == 0:
                    return arr[f"x0c{min(d0 // 2, 3)}"]
                return arr[f"x{qb}"]

            # projection chains, in deadline order
            chains = []       # list of (chain_id, [(cost, fn, ready), ...])
            chain_ix = {}

            def add_chain(cid, items):
                chain_ix[cid] = len(chains)
                chains.append((cid, items))

            def add_kq(qb, tt, wkey):
                add_chain((qb, f"e{tt}"), [
                    (c, f, max(arr[wkey], xavail(qb, 2 * i)))
                    for i, (c, f) in enumerate(
                        proj_kq_items(qb, tt, xqbs[qb]))
                ])

            # deadline order: this block's hp1 weights, next block's hp0
            # weights, THEN this block's v (only PV chunks wait on v, and
            # they gate on the v chain explicitly).
            add_kq(0, 0, "wkqA")
            add_kq(0, 2, "wkqA")
            for qb in range(NQB):
                add_kq(qb, 1, "wkqB")
                add_kq(qb, 3, "wkqB")
                if qb + 1 < NQB:
                    add_kq(qb + 1, 0, "wkqA")
                    add_kq(qb + 1, 2, "wkqA")
                add_chain((qb, "v"), [
                    (c, f, max(arr["wv"], xavail(qb, 4 * (i % 2))))
                    for i, (c, f) in enumerate(
                        proj_v_items(qb, xqbs[qb]))
                ])
            pos = [0] * len(chains)        # next item per chain
            started = [False] * len(chains)

            def chain_done(cid):
                i = chain_ix[cid]
                return pos[i] >= len(chains[i][1])

            pv_open = [0]  # pv chains currently holding an mm_ps buf

            def inflight_count():
                return pv_open[0] + sum(
                    1 for i in range(len(chains))
                    if started[i] and pos[i] < len(chains[i][1])
                )

            def next_proj(now):
                # continue an in-flight chain if its next item is ready;
                # else start the earliest pending chain (<=2 in flight)
                for i in range(len(chains)):
                    if started[i] and pos[i] < len(chains[i][1]):
                        if chains[i][1][pos[i]][2] <= now:
                            return i
                if inflight_count() < 2:
                    for i in range(len(chains)):
                        if not started[i]:
                            if chains[i][1][0][2] <= now:
                                return i
                            break  # deadline order: don't skip ahead far
                return None

            def any_proj_left():
                return any(pos[i] < len(chains[i][1])
                           for i in range(len(chains)))

            def force_proj():
                # emit the earliest unfinished chain item (may stall PE)
                for i in range(len(chains)):
                    if pos[i] < len(chains[i][1]):
                        if started[i] or inflight_count() < 2:
                            return i
                return None

            pend = deque()  # (need_g, vchain, cost, fn) pv chunks
            stocked = set()

            def stock_pv(qb, hp):
                base = sc_start[(qb, hp)]
                for cost, fn, maxkt, first, last in pv_items(
                    qb, hp, osbs[qb]
                ):
                    need = base + maxkt
                    vc = (qb, "v") if maxkt >= 4 * qb else None
                    pend.append((need, vc, cost, fn, first, last))

            pe_t = 3200.0
            exp_end = [0.0] * NSC
            g = 0

            def bb_ok():
                if g >= NSC:
                    return False
                qb, hp, kt, _, _, _ = backbone[g]
                if hp == 0:
                    if not (chain_done((qb, "e0"))
                            and chain_done((qb, "e2"))):
                        return False
                    # SC0(qb) exps overwrite pt0[qb%2]: every PV0(qb-2)
                    # chunk (same parity plane) must already be emitted,
                    # or its later reads would silently see the new data.
                    if qb >= 2:
                        thr = sc_start[(qb - 2, 1)]
                        if any(n < thr for n, *_ in pend):
                            return False
                    return True
                if not (chain_done((qb, "e1")) and chain_done((qb, "e3"))):
                    return False
                # SC1(qb) exps overwrite pt1 (no parity): every PV1(qb-1)
                # chunk must already be emitted.
                if qb >= 1:
                    thr = sc_start[(qb, 0)]
                    if any(n < thr for n, *_ in pend):
                        return False
                return True

            def emit_sc():
                nonlocal pe_t, g
                qb, hp, kt, mm_ns, exp_ns, diag = backbone[g]
                if (qb, hp) not in stocked:
                    stocked.add((qb, hp))
                    stock_pv(qb, hp)
                if g >= 3:
                    pe_t = max(pe_t, exp_end[g - 3] - 500.0)
                sc(qb, hp, kt)
                pe_t += mm_ns
                prev = exp_end[g - 1] if g else 0.0
                exp_end[g] = max(prev, pe_t + 200.0) + exp_ns
                if diag:
                    exp_end[g] += MASK_NS
                g += 1

            def emit_chain(i):
                nonlocal pe_t
                cost, fn, ready = chains[i][1][pos[i]]
                fn()
                started[i] = True
                pos[i] += 1
                pe_t = max(pe_t, ready) + cost

            def pv_head_ready():
                if not pend:
                    return False
                need, vc, _, _, first, _ = pend[0]
                if need >= g:
                    return False
                if vc is not None and not chain_done(vc):
                    return False
                if first and inflight_count() >= 2:
                    return False
                return exp_end[need] <= pe_t

            def pv_head_emittable():
                if not pend:
                    return False
                need, vc, _, _, first, _ = pend[0]
                if need >= g:  # its exp is not even emitted yet
                    return False
                if vc is not None and not chain_done(vc):
                    return False
                return not first or inflight_count() < 2

            def emit_pv():
                nonlocal pe_t
                need, vc, cost, fn, first, last = pend.popleft()
                fn()
                if first:
                    pv_open[0] += 1
                if last:
                    pv_open[0] -= 1
                pe_t = max(pe_t, exp_end[need]) + cost

            out_emitted = [False] * NQB

            def maybe_out():
                for qb in range(NQB - 1):
                    if out_emitted[qb]:
                        continue
                    last_g = sc_start[(qb, 1)] + 4 * (qb + 1) - 1
                    if g <= last_g:
                        continue
                    if any(n <= last_g for n, *_ in pend):
                        continue
                    out_emitted[qb] = True
                    nc.sync.dma_start(
                        o_v[:, qb * 4:(qb + 1) * 4, :], osbs[qb][:, :, :]
                    )

            while g < NSC or any_proj_left() or pend:
                backlog = (exp_end[g - 1] - pe_t) if g else 0.0
                if bb_ok() and backlog < BACKLOG:
                    emit_sc()
                else:
                    i = next_proj(pe_t)
                    if i is not None:
                        emit_chain(i)
                    elif pv_head_ready():
                        emit_pv()
                    elif bb_ok():
                        emit_sc()
                    elif pv_head_emittable():
                        emit_pv()  # stalls PE on the exp, but nothing else
                    else:
                        i = force_proj()
                        if i is not None:
                            emit_chain(i)
                        elif pv_head_emittable():
                            emit_pv()
                        else:
                            emit_sc()
                maybe_out()
            maybe_out()
            for qb in range(NQB - 1):
                if not out_emitted[qb]:
                    nc.sync.dma_start(
                        o_v[:, qb * 4:(qb + 1) * 4, :], osbs[qb][:, :, :]
                    )

    if split_waits:
        _split_matmul_waits(nc)
    return nc


_nc_cache = None


def _get_nc():
    global _nc_cache
    if _nc_cache is None:
        _nc_cache = build_nc()
    return _nc_cache


def make_in_maps(x, W, b):
    x = np.asarray(x, dtype=np.float32)
    W = np.asarray(W, dtype=np.float32)
    b = np.asarray(b, dtype=np.float32)
    in_maps = []
    xTs = [np.ascontiguousarray(x[n].T.astype(np.float16)) for n in range(N)]
    for c in range(8):
        n, g = divmod(c, 4)
        rk = slice(256 * g, 256 * g + 256)
        rq = slice(D + 256 * g, D + 256 * g + 256)
        rv = slice(2 * D + 256 * g, 2 * D + 256 * g + 256)
        wkqm = np.ascontiguousarray(
            np.concatenate(
                [W[rk][:P], W[rq][:P], W[rk][P:], W[rq][P:]], axis=0
            ).T.astype(np.float16)
        )
        wvm = np.ascontiguousarray(W[rv].T.astype(np.float16))
        bkqm = np.ascontiguousarray(
            np.stack(
                [b[rk][:P], b[rq][:P], b[rk][P:], b[rq][P:]], axis=0
            ).T
        )
        bvm = np.ascontiguousarray(b[rv].reshape(1, EV))
        in_maps.append(
            {"xT": xTs[n], "wkq": wkqm, "wv": wvm, "bkq": bkqm, "bv": bvm}
        )
    return in_maps


def run(inputs, **kwargs):
    nc = _get_nc()
    in_maps = make_in_maps(inputs["x"], inputs["W"], inputs["b"])
    res = run_bass_kernel_spmd(nc, in_maps, core_ids=list(range(8)), **kwargs)
    out = np.empty((N, S, D), dtype=np.float32)
    for c in range(8):
        n, g = divmod(c, 4)
        out[n, :, 256 * g:256 * g + 256] = res.results[c]["o"]
    return out, res


def kernel(**inputs):
    out, _ = run(inputs)
    return out


# revision 31
# speedup vs baseline: 1.3160x; 1.1827x over previous
"""Masked multi-head attention (fused QKV) on 8 trn2 NeuronCores.

Problem (full shapes): x [2, 2048, 1024] f32, W [3072, 1024], b [3072].
  z = x @ W.T + b ; k,q,v = split(z) ; heads H=16, hd=64
  out = softmax(causal(q k^T / sqrt(1024))) v   -> [2, 2048, 1024]

Sharding: core c handles batch n=c//4 and head group g=c%4 (4 heads).
Each core is fully independent (data + head parallel, no collectives).
The host pre-transposes x[n] and the per-core W slices (cast to fp16);
results are sliced back into out[n, :, 256g:256g+256].

Per-core device program (all matmuls fp16 in / f32 PSUM accumulate):
  1) v natural [seq, 4*64] = matmul(lhsT=xT tile, rhs=WvT); bias added on
     the DVE evacuation; stored fp16 as [128, ktile, head, 65] with a
     ones column fused in for the softmax denominator.
  2) k,q transposed: zT e-tiles [128, seq] = matmul(lhsT=WkqT tile,
     rhs=xT tile); per-partition bias added on the DVE evacuation (fp16
     out). Each e-tile holds an even/odd head pair on partitions
     0:64/64:128.
  3) Per (q-block 512, head pair): S^T k-tiles [128, w<=512] via K=64
     matmuls at partition bases 0/64. One ACT exp (scale=1/32, no max
     subtraction needed) per k-tile evacuates both heads' PSUM banks
     through a strided [128, 2, w] AP into fp16 pt. Causal masking
     touches only the 4 diagonal k-tiles (DVE multiply by a triangle
     mask, both heads in one instruction), and fully-masked columns
     (< 128r on diagonal tile r) are trimmed everywhere.
  4) PV flipped: out[q, d] accumulated in PSUM as [128 q, 65] tiles with
     lhsT = S^T slice [128 kpos, 128 q] (stationary; ldweights is free)
     and rhs = [V | 1] [128 kpos, 65] (moving, only 65 columns billed).
     Column 64 = sum of exp. Output lands in natural [seq, e] layout --
     no transposes. DVE reciprocal of column 64 + broadcast multiply
     normalize straight into the output staging tile.

Scheduling: the serial ACT exp chain (~1 us per k-tile) paces the
attention stream while each score matmul pair costs the PE only ~0.4 us,
and the PE issues strictly in order. The emitter therefore interleaves
"filler" PE work -- projection d-tile steps for the next q-block and PV
accumulation chunks once their exponentials have landed -- between score
tiles, budgeted per gap, so neither PE nor ACT ever waits on the other.

_split_matmul_waits() is a required legalization for this compiler
build: every engine instruction may carry at most one semaphore wait.
"""

from collections import deque

import numpy as np

import concourse.bass as bass
import concourse.mybir as mybir
import concourse.tile as tile
from concourse.bass_utils import run_bass_kernel_spmd

F32 = mybir.dt.float32
F16 = mybir.dt.float16

N, S, D = 2, 2048, 1024
H, HD = 16, 64
P = 128
QB = 512                 # q block (free dim per matmul)
NQB = S // QB            # 4
NKT = S // P             # 16 k tiles
ND = D // P              # 8 contraction tiles
NHC = 4                  # heads per core
EKQ = 2 * NHC * HD       # 512 = k+q rows per core
EV = NHC * HD            # 256 = v rows per core
SCALE = 1.0 / 32.0       # 1/sqrt(1024)
PE_NS = 0.4167           # ns per PE row at full p-state

AF = mybir.ActivationFunctionType
ALU = mybir.AluOpType


def _split_matmul_waits(nc):
    """Move semaphore waits off Matmult instructions onto preceding PE NOPs.

    The walrus codegen for self-loading fp32/fp32r matmuls folds waits into
    the LDWEIGHTS struct, which has room for a single sync-wait command;
    two producers (e.g. two DMA queues) make it fail with "Too many sync
    wait commands". Sequencer NOPs on the same engine execute in program
    order, so hoisting each wait onto its own NOP is semantics-preserving.
    """
    import bass_rust

    moved = 0
    for bb in nc.main_func.blocks:
        out = []
        for ins in bb.instructions:
            si = ins.sync_info
            keep = 0 if isinstance(ins, bass_rust.InstMatmult) else 1
            if (
                not isinstance(ins, bass_rust.InstNoOp)
                and si is not None
                and len(si.on_wait) > keep
            ):
                hoist = si.on_wait[keep:] if keep else si.on_wait
                for j, w in enumerate(hoist):
                    out.append(
                        bass_rust.InstNoOp(
                            name=f"{ins.name}-hw{j}",
                            engine=ins.engine,
                            sync_info=mybir.SyncInfo(on_wait=[w], on_update=[]),
                        )
                    )
                    moved += 1
                ins.sync_info = mybir.SyncInfo(
                    on_wait=list(si.on_wait[:keep]), on_update=list(si.on_update)
                )
            out.append(ins)
        bb.instructions[:] = out
    return moved


def build_nc(split_waits=True):
    nc = bass.Bass()

    xT = nc.dram_tensor("xT", [D, S], F16, kind="ExternalInput")
    wkq = nc.dram_tensor("wkq", [D, EKQ], F16, kind="ExternalInput")
    wv = nc.dram_tensor("wv", [D, EV], F16, kind="ExternalInput")
    bkq = nc.dram_tensor("bkq", [P, 4], F32, kind="ExternalInput")
    bv = nc.dram_tensor("bv", [1, EV], F32, kind="ExternalInput")
    o = nc.dram_tensor("o", [S, EV], F32, kind="ExternalOutput")

    xT_v = xT.rearrange("(dt p) s -> p dt s", p=P)       # [128, 8, 2048]
    o_v = o.rearrange("(qt p) c -> p qt c", p=P)         # [128, 16, 256]

    with tile.TileContext(nc) as tc:
        with (
            tc.tile_pool(name="const", bufs=1) as const,
            tc.tile_pool(name="big", bufs=1) as big,
            tc.tile_pool(name="xpool", bufs=3) as xpool,
            tc.tile_pool(name="work", bufs=2) as work,
            tc.tile_pool(name="opool", bufs=4) as opool,
            tc.tile_pool(name="mm_ps", bufs=2, space="PSUM") as mm_ps,
            tc.tile_pool(name="st_ps", bufs=3, space="PSUM") as st_ps,
        ):
            # ---- input DMAs. Host lays wkq columns out as
            # [k01 | q01 | k23 | q23] so head pair 0's weights (one 512B-
            # contiguous chunk) can land first; x streams in 2-dtile chunks
            # matching the projection item granularity, so the first
            # matmuls start ~3.5us in instead of waiting ~9us for
            # everything.
            # Issue overheads spread over three HWDGE queues (SP, ACT,
            # DVE are all idle at start); transfers serialize through the
            # shared DMA engines in issue order, so the big non-critical
            # loads (x1..x3, wv) queue on SP *behind* the startup set.
            xqbs = [None] * NQB
            wkq_v = wkq.rearrange("(dt p) e -> p dt e", p=P)
            wkq_sb = const.tile([P, ND, EKQ], F16)
            bkq_sb = const.tile([P, 4], F32)
            wv_sb = const.tile([P, ND, EV], F16)
            bvb = const.tile([P, EV], F32)
            for i in range(NQB):
                xqbs[i] = xpool.tile(
                    [P, ND, QB], F16, tag="xqb", name=f"xqb{i}"
                )
            nc.sync.dma_start(wkq_sb[:, :, 0:EKQ // 2], wkq_v[:, :, 0:EKQ // 2])
            nc.scalar.dma_start(
                wkq_sb[:, :, EKQ // 2:], wkq_v[:, :, EKQ // 2:]
            )
            nc.scalar.dma_start(bkq_sb, bkq[:, :])
            nc.scalar.dma_start(bvb, bv[:, :].partition_broadcast(P))
            for dc in range(0, ND, 2):
                nc.sync.dma_start(
                    xqbs[0][:, dc:dc + 2], xT_v[:, dc:dc + 2, 0:QB]
                )
            nc.sync.dma_start(xqbs[1], xT_v[:, :, QB:2 * QB])
            nc.sync.dma_start(wv_sb, wv.rearrange("(dt p) e -> p dt e", p=P))
            nc.sync.dma_start(xqbs[2], xT_v[:, :, 2 * QB:3 * QB])
            nc.sync.dma_start(xqbs[3], xT_v[:, :, 3 * QB:4 * QB])

            # ---- constants ----
            onef = const.tile([P, 1], F32)
            nc.vector.memset(onef, 1.0)
            # warm the ACT exp table while DMAs run
            dummy = const.tile([1, 2], F32)
            nc.gpsimd.memset(dummy, 0.0)
            nc.scalar.activation(dummy, dummy, AF.Exp)

            # ---- persistent state ----
            # zT for k,q: e-tiles 0,1 = [k_h0;k_h1],[k_h2;k_h3]; 2,3 = q same
            zkq = big.tile([P, 4, S], F16)
            # v natural + ones column: [p, ktile, head, 65]
            vsb = big.tile([P, NKT, NHC, HD + 1], F16)
            nc.vector.tensor_copy(
                vsb[:, :, :, HD:HD + 1],
                onef[:, :, None].to_broadcast((P, NKT, NHC, 1)),
            )  # ones column for the fused sum(exp) row
            # diagonal causal mask: mask[p, q] = 1 if q >= p (same for every
            # diagonal tile after its dead columns are trimmed)
            mask32 = const.tile([P, QB], F32)
            nc.gpsimd.affine_select(
                out=mask32,
                in_=onef.to_broadcast((P, QB)),
                compare_op=ALU.is_ge, fill=0.0,
                base=0, channel_multiplier=-1,
                pattern=[[1, QB]],
            )
            mask16 = const.tile([P, QB], F16)
            nc.vector.tensor_copy(mask16, mask32)
            # exp(S^T): both head pairs double-buffer their plane by
            # q-block parity so PV(qb) can keep draining while qb+1's exps
            # land. The kt axis is packed: parity plane 0 (qb 0/2) needs
            # only 12 k-tiles, plane 1 (qb 1/3) needs 16 -> 28 slots.
            # layout: ptX [p, head, packed ktile, q]; plane base = 0 or 12.
            NKP = 28
            pt0 = big.tile([P, 2, NKP, QB], F16)
            pt1 = big.tile([P, 2, NKP, QB], F16)

            # ---- filler queue: (est_pe_ns, emit_fn) ----
            fill = deque()

            def run_fill(budget):
                while fill and budget > 0:
                    cost, fn = fill.popleft()
                    fn()
                    budget -= cost

            def drain_fill():
                run_fill(float("inf"))

            ESLOT = {0: 0, 2: 1, 1: 2, 3: 3}  # e-tile -> host column block

            def proj_kq_items(qb, t, xqb):
                # zT e-tile t for q-block qb, split into 2-dtile steps
                state = {}
                s = ESLOT[t]

                def step(d0, first, last):
                    def fn():
                        if first:
                            state["ps"] = mm_ps.tile(
                                [P, QB], F32, tag="mmps", name="kqps"
                            )
                        ps = state["ps"]
                        for dt in range(d0, d0 + 2):
                            nc.tensor.matmul(
                                ps,
                                lhsT=(wkq_sb[:, dt, s * P:(s + 1) * P]),
                                rhs=(xqb[:, dt, :]),
                                start=(dt == 0), stop=(dt == ND - 1),
                            )
                        if last:
                            nc.vector.tensor_scalar_add(
                                zkq[:, t, qb * QB:(qb + 1) * QB],
                                ps, bkq_sb[:, s:s + 1],
                            )
                    return fn

                return [
                    (2 * QB * PE_NS, step(d0, d0 == 0, d0 == ND - 2))
                    for d0 in range(0, ND, 2)
                ]

            def proj_v_items(qb, xqb):
                # v natural for the 4 q-tiles of qb, 4-dtile steps
                items = []
                for qt4 in range(4):
                    qt = qb * 4 + qt4
                    state = {}

                    def step(d0, first, last, qt=qt, qt4=qt4, state=state):
                        def fn():
                            if first:
                                state["ps"] = mm_ps.tile(
                                    [P, QB], F32, tag="mmps", name="vps"
                                )
                            ps = state["ps"]
                            for dt in range(d0, d0 + 4):
                                nc.tensor.matmul(
                                    ps[:, :EV],
                                    lhsT=(xqb[:, dt, qt4 * P:(qt4 + 1) * P]),
                                    rhs=(wv_sb[:, dt, :]),
                                    start=(dt == 0), stop=(dt == ND - 1),
                                )
                            if last:
                                nc.vector.tensor_tensor(
                                    vsb[:, qt, :, 0:HD],
                                    ps[:, :EV].rearrange(
                                        "p (h d) -> p h d", d=HD
                                    ),
                                    bvb.rearrange("p (h d) -> p h d", d=HD),
                                    mybir.AluOpType.add,
                                )
                        return fn

                    for d0 in range(0, ND, 4):
                        items.append(
                            (4 * EV * PE_NS, step(d0, d0 == 0, d0 == ND - 4))
                        )
                return items

            def pv_items(qb, hp, osb):
                # flipped PV + normalize for head pair hp of q-block qb;
                # each chunk carries the max k-tile whose exp it needs so
                # the emitter can flow it into the score loop as soon as
                # that exp has retired.
                items = []
                for qt4 in range(4):
                    nkt = 4 * qb + qt4 + 1
                    mms = [(kt, hl) for kt in range(nkt) for hl in range(2)]
                    state = {}
                    CH = 8
                    chunks = [mms[i:i + CH] for i in range(0, len(mms), CH)]
                    for ci, chunk in enumerate(chunks):
                        first = ci == 0
                        last = ci == len(chunks) - 1

                        def fn(chunk=chunk, first=first, last=last,
                               qt4=qt4, nkt=nkt, state=state):
                            if first:
                                state["pvt"] = mm_ps.tile(
                                    [P, 2, HD + 1], F32, tag="mmps",
                                    name="pvt"
                                )
                            pvt = state["pvt"]
                            src_pt = pt0 if hp == 0 else pt1
                            kb = 12 if qb % 2 else 0
                            for kt, hl in chunk:
                                nc.tensor.matmul(
                                    pvt[:, hl, :],
                                    lhsT=src_pt[:, hl, kb + kt,
                                                qt4 * P:(qt4 + 1) * P],
                                    rhs=vsb[:, kt, 2 * hp + hl, :],
                                    start=(kt == 0 and hl == 0),
                                    stop=(kt == nkt - 1 and hl == 1),
                                    skip_group_check=True,
                                )
                            if last:
                                rs = work.tile([P, 2], F32, tag="rs")
                                nc.vector.reciprocal(rs, pvt[:, :, HD])
                                nc.vector.tensor_tensor(
                                    osb[:, qt4, 2 * hp * HD:(2 * hp + 2) * HD]
                                    .rearrange("p (h d) -> p h d", d=HD),
                                    pvt[:, :, 0:HD],
                                    rs[:, :, None].to_broadcast((P, 2, HD)),
                                    mybir.AluOpType.mult,
                                )
                                if qb == NQB - 1 and hp == 1:
                                    dq = nc.sync if qt4 % 2 == 0 else nc.scalar
                                    dq.dma_start(
                                        o_v[:, qb * 4 + qt4, :],
                                        osb[:, qt4, :],
                                    )
                        maxkt = max(kt for kt, _ in chunk)
                        items.append(
                            (len(chunk) * (HD + 1) * PE_NS, fn, maxkt,
                             first, last)
                        )
                return items

            def sc(qb, hp, kt):
                # score matmul pair + exp (+ diagonal mask); returns the
                # filler budget this k-tile's exp buys on the PE.
                r = kt - 4 * qb
                off = P * r if 0 <= r < 4 else 0
                w = QB - off
                stp = st_ps.tile([P, 2, QB], F32, tag="st")
                for hl in range(2):
                    base = HD * hl
                    nc.tensor.matmul(
                        stp[:, hl, 0:w],
                        lhsT=zkq[base:base + HD, hp, kt * P:(kt + 1) * P],
                        rhs=zkq[base:base + HD, 2 + hp,
                                qb * QB + off:(qb + 1) * QB],
                        start=True, stop=True,
                    )
                dst = pt0 if hp == 0 else pt1
                ktp = (12 if qb % 2 else 0) + kt
                nc.scalar.activation(
                    dst[:, :, ktp, off:QB],
                    stp[:, :, 0:w],
                    AF.Exp, scale=SCALE,
                )
                if 0 <= r < 4:
                    # gpsimd, not DVE: keeps the DVE wait queue free of
                    # ops gated on late exps (head-of-line blocking). The
                    # final block's hp1 masks go to the (then idle, 4x
                    # faster) DVE since they sit on the drain critical path.
                    eng = (
                        nc.vector
                        if (qb == NQB - 1 and hp == 1) else nc.gpsimd
                    )
                    eng.tensor_tensor(
                        out=dst[:, :, ktp, off:QB],
                        in0=dst[:, :, ktp, off:QB],
                        in1=mask16[:, None, 0:w].to_broadcast((P, 2, w)),
                        op=mybir.AluOpType.mult,
                    )
                # exp busy (0.833/row + overhead) minus this pair's PE time
                return 2 * w * (0.8333 - PE_NS) + 185.0

            # ---- model-driven emission ----
            # A small list scheduler with virtual PE/ACT clocks decides, at
            # every step, whether to emit the next backbone score tile
            # (keeping the serial ACT exp chain fed) or a filler item:
            # projection chain steps (deadline order, gated on a DMA-arrival
            # model, at most two chains in flight for the 2 PSUM bufs) or PV
            # chunks gated on modeled exp completion. This keeps PE and ACT
            # dense through every q-block boundary without hand-tuned drains.
            osbs = [None] * NQB
            for qb in range(NQB):
                osbs[qb] = opool.tile(
                    [P, 4, EV], F32, tag="osb", name=f"osb{qb}"
                )

            EXP_NS = 0.8333
            EXP_OH = 185.0
            MASK_NS = 300.0
            BACKLOG = 2200.0
            EVAC_LAT = 800.0

            backbone = []  # (qb, hp, kt, mm_ns, exp_ns, diag)
            for qb in range(NQB):
                for hp in range(2):
                    for kt in range(4 * (qb + 1)):
                        r = kt - 4 * qb
                        w = QB - (P * r if 0 <= r < 4 else 0)
                        backbone.append(
                            (qb, hp, kt, 2 * w * PE_NS,
                             2 * w * EXP_NS + EXP_OH, 0 <= r < 4)
                        )
            NSC = len(backbone)
            sc_start = {}
            gidx = 0
            for qb in range(NQB):
                for hp in range(2):
                    sc_start[(qb, hp)] = gidx
                    gidx += 4 * (qb + 1)

            # DMA arrival model (ns): merged transfer order across the
            # three issue queues above, serialized through the DMA engines.
            arr = {
                "wkqA": 3200.0, "wkqB": 4700.0, "bkq": 4750.0,
                "bv": 4800.0, "x0c0": 5500.0, "x0c1": 6300.0,
                "x0c2": 7000.0, "x0c3": 7700.0, "x1": 10600.0,
                "wv": 12100.0, "x2": 15000.0, "x3": 17900.0,
            }

            def xavail(qb, d0):
                if qb == 0:
                    return arr[f"x0c{min(d0 // 2, 3)}"]
                return arr[f"x{qb}"]

            # projection chains, in deadline order
            chains = []       # list of (chain_id, [(cost, fn, ready), ...])
            chain_ix = {}

            def add_chain(cid, items):
                chain_ix[cid] = len(chains)
                chains.append((cid, items))

            def add_kq(qb, tt, wkey):
                add_chain((qb, f"e{tt}"), [
                    (c, f, max(arr[wkey], xavail(qb, 2 * i)))
                    for i, (c, f) in enumerate(
                        proj_kq_items(qb, tt, xqbs[qb]))
                ])

            # deadline order: this block's hp1 weights, next block's hp0
            # weights, THEN this block's v (only PV chunks wait on v, and
            # they gate on the v chain explicitly).
            def add_v(qb):
                add_chain((qb, "v"), [
                    (c, f, max(arr["wv"], xavail(qb, 4 * (i % 2))))
                    for i, (c, f) in enumerate(
                        proj_v_items(qb, xqbs[qb]))
                ])

            # deadline order (backbone index where each chain is required):
            # e0/e2(qb) at SC0(qb), e1/e3(qb) at SC1(qb), v(qb) loosely
            # before the PV drain barrier of its block.
            add_kq(0, 0, "wkqA")
            add_kq(0, 2, "wkqA")
            add_kq(0, 1, "wkqB")
            add_kq(0, 3, "wkqB")
            add_kq(1, 0, "wkqA")
            add_kq(1, 2, "wkqA")
            add_kq(1, 1, "wkqB")
            add_kq(1, 3, "wkqB")
            add_v(0)
            add_kq(2, 0, "wkqA")
            add_kq(2, 2, "wkqA")
            add_v(1)
            add_kq(2, 1, "wkqB")
            add_kq(2, 3, "wkqB")
            add_kq(3, 0, "wkqA")
            add_kq(3, 2, "wkqA")
            add_v(2)
            add_kq(3, 1, "wkqB")
            add_kq(3, 3, "wkqB")
            add_v(3)
            pos = [0] * len(chains)        # next item per chain
            started = [False] * len(chains)

            def chain_done(cid):
                i = chain_ix[cid]
                return pos[i] >= len(chains[i][1])

            pv_open = [0]  # pv chains currently holding an mm_ps buf

            def inflight_count():
                return pv_open[0] + sum(
                    1 for i in range(len(chains))
                    if started[i] and pos[i] < len(chains[i][1])
                )

            def next_proj(now):
                # continue an in-flight chain if its next item is ready;
                # else start the earliest pending chain -- but leave a PSUM
                # slot free when the head PV chain is waiting for one.
                for i in range(len(chains)):
                    if started[i] and pos[i] < len(chains[i][1]):
                        if chains[i][1][pos[i]][2] <= now:
                            return i
                cap = 1 if pv_wants_slot() else 2
                if inflight_count() < cap:
                    for i in range(len(chains)):
                        if not started[i]:
                            if chains[i][1][0][2] <= now:
                                return i
                            break  # deadline order: don't skip ahead far
                return None

            def any_proj_left():
                return any(pos[i] < len(chains[i][1])
                           for i in range(len(chains)))

            def force_proj():
                # emit the earliest unfinished chain item (may stall PE)
                for i in range(len(chains)):
                    if pos[i] < len(chains[i][1]):
                        if started[i] or inflight_count() < 2:
                            return i
                return None

            pend = deque()  # (need_g, vchain, cost, fn) pv chunks
            stocked = set()

            def stock_pv(qb, hp):
                base = sc_start[(qb, hp)]
                items = pv_items(qb, hp, osbs[qb])
                tail = qb == NQB - 1 and hp == 1
                # a chain holds one of the two shared PSUM bufs from first
                # chunk to normalize; except at the very tail, don't open
                # one until every exp it needs has retired, so it never
                # camps on the buf and blocks projection chains.
                chain_last = {}
                if not tail:
                    for _, _, maxkt, first, _ in items:
                        if first:
                            cur = []
                            chain_last[id(cur)] = None
                    lasts = []
                    cur_last = 0
                    for cost, fn, maxkt, first, last in items:
                        cur_last = max(cur_last, maxkt)
                        if last:
                            lasts.append(cur_last)
                            cur_last = 0
                    ci = -1
                    out = []
                    for cost, fn, maxkt, first, last in items:
                        if first:
                            ci += 1
                        gate = maxkt if tail else lasts[ci]
                        out.append((cost, fn, gate, first, last))
                    items = out
                else:
                    items = [
                        (cost, fn, maxkt, first, last)
                        for cost, fn, maxkt, first, last in items
                    ]
                for cost, fn, gatekt, first, last in items:
                    need = base + gatekt
                    vc = (qb, "v") if gatekt >= 4 * qb else None
                    pend.append((need, vc, cost, fn, first, last))

            pe_t = 3200.0
            exp_end = [0.0] * NSC
            g = 0

            def bb_ok():
                if g >= NSC:
                    return False
                qb, hp, kt, _, _, _ = backbone[g]
                if hp == 0:
                    if not (chain_done((qb, "e0"))
                            and chain_done((qb, "e2"))):
                        return False
                    # SC0(qb) exps overwrite pt0's qb%2 plane: every
                    # PV0(qb-2) chunk (same plane) must already be emitted,
                    # or its later reads would silently see the new data.
                    if qb >= 2:
                        thr = sc_start[(qb - 2, 1)]
                        if any(n < thr for n, *_ in pend):
                            return False
                    return True
                if not (chain_done((qb, "e1")) and chain_done((qb, "e3"))):
                    return False
                # SC1(qb) exps overwrite pt1's qb%2 plane: every PV1(qb-2)
                # chunk must already be emitted.
                if qb >= 2:
                    thr = sc_start[(qb - 1, 0)]
                    if any(n < thr for n, *_ in pend):
                        return False
                return True

            def ramp(c):
                return c * 2.0 if pe_t < 7000.0 else c

            def emit_sc():
                nonlocal pe_t, g
                qb, hp, kt, mm_ns, exp_ns, diag = backbone[g]
                if (qb, hp) not in stocked:
                    stocked.add((qb, hp))
                    stock_pv(qb, hp)
                if kt == 0:
                    eps = ("e0", "e2") if hp == 0 else ("e1", "e3")
                    for e in eps:
                        pe_t = max(
                            pe_t, chain_fin.get((qb, e), 0.0) + EVAC_LAT
                        )
                if g >= 3:
                    pe_t = max(pe_t, exp_end[g - 3] - 500.0)
                sc(qb, hp, kt)
                pe_t += ramp(mm_ns)
                prev = exp_end[g - 1] if g else 0.0
                exp_end[g] = max(prev, pe_t + 200.0) + exp_ns
                if diag:
                    exp_end[g] += MASK_NS
                g += 1

            chain_fin = {}

            def emit_chain(i):
                nonlocal pe_t
                cost, fn, ready = chains[i][1][pos[i]]
                fn()
                started[i] = True
                pos[i] += 1
                pe_t = max(pe_t, ready) + ramp(cost)
                if pos[i] >= len(chains[i][1]):
                    chain_fin[chains[i][0]] = pe_t

            def pv_wants_slot():
                if not pend:
                    return False
                need, vc, _, _, first, _ = pend[0]
                return (
                    first and need < g
                    and (vc is None or chain_done(vc))
                )

            def pv_head_ready():
                if not pend:
                    return False
                need, vc, _, _, first, _ = pend[0]
                if need >= g:
                    return False
                if vc is not None and not chain_done(vc):
                    return False
                if first and inflight_count() >= 2:
                    return False
                return exp_end[need] <= pe_t

            def pv_head_emittable():
                if not pend:
                    return False
                need, vc, _, _, first, _ = pend[0]
                if need >= g:  # its exp is not even emitted yet
                    return False
                if vc is not None and not chain_done(vc):
                    return False
                return not first or inflight_count() < 2

            def emit_pv():
                nonlocal pe_t
                need, vc, cost, fn, first, last = pend.popleft()
                fn()
                if first:
                    pv_open[0] += 1
                if last:
                    pv_open[0] -= 1
                pe_t = max(pe_t, exp_end[need]) + cost

            out_emitted = [False] * NQB

            def maybe_out():
                for qb in range(NQB - 1):
                    if out_emitted[qb]:
                        continue
                    last_g = sc_start[(qb, 1)] + 4 * (qb + 1) - 1
                    if g <= last_g:
                        continue
                    if any(n <= last_g for n, *_ in pend):
                        continue
                    out_emitted[qb] = True
                    nc.sync.dma_start(
                        o_v[:, qb * 4:(qb + 1) * 4, :], osbs[qb][:, :, :]
                    )

            import os
            elog = [] if os.environ.get("KLOG") else None

            def log(kind, detail=""):
                if elog is not None:
                    elog.append(f"{pe_t/1000:8.2f} g={g:2d} {kind} {detail}")

            while g < NSC or any_proj_left() or pend:
                backlog = (exp_end[g - 1] - pe_t) if g else 0.0
                if bb_ok() and backlog < BACKLOG:
                    log("sc", backbone[g][:3])
                    emit_sc()
                else:
                    if pv_head_ready():
                        log("pv", pend[0][0])
                        emit_pv()
                        maybe_out()
                        continue
                    i = next_proj(pe_t)
                    if i is not None:
                        log("proj", chains[i][0])
                        emit_chain(i)
                    elif bb_ok():
                        log("sc2", backbone[g][:3])
                        emit_sc()
                    elif pv_head_emittable():
                        log("pv-stall", pend[0][0])
                        emit_pv()  # stalls PE on the exp, but nothing else
                    else:
                        i = force_proj()
                        if i is not None:
                            log("proj-f", chains[i][0])
                            emit_chain(i)
                        elif pv_head_emittable():
                            log("pv-f", pend[0][0])
                            emit_pv()
                        else:
                            log("sc-f", backbone[g][:3] if g < NSC else "END")
                            emit_sc()
                maybe_out()
            if elog is not None:
                with open("/tmp/emission.log", "w") as f:
                    f.write("\n".join(elog))
            maybe_out()
            for qb in range(NQB - 1):
                if not out_emitted[qb]:
                    nc.sync.dma_start(
                        o_v[:, qb * 4:(qb + 1) * 4, :], osbs[qb][:, :, :]
                    )

    if split_waits:
        _split_matmul_waits(nc)
    return nc


_nc_cache = None


def _get_nc():
    global _nc_cache
    if _nc_cache is None:
        _nc_cache = build_nc()
    return _nc_cache


def make_in_maps(x, W, b):
    x = np.asarray(x, dtype=np.float32)
    W = np.asarray(W, dtype=np.float32)
    b = np.asarray(b, dtype=np.float32)
    in_maps = []
    xTs = [np.ascontiguousarray(x[n].T.astype(np.float16)) for n in range(N)]
    for c in range(8):
        n, g = divmod(c, 4)
        rk = slice(256 * g, 256 * g + 256)
        rq = slice(D + 256 * g, D + 256 * g + 256)
        rv = slice(2 * D + 256 * g, 2 * D + 256 * g + 256)
        wkqm = np.ascontiguousarray(
            np.concatenate(
                [W[rk][:P], W[rq][:P], W[rk][P:], W[rq][P:]], axis=0
            ).T.astype(np.float16)
        )
        wvm = np.ascontiguousarray(W[rv].T.astype(np.float16))
        bkqm = np.ascontiguousarray(
            np.stack(
                [b[rk][:P], b[rq][:P], b[rk][P:], b[rq][P:]], axis=0
            ).T
        )
        bvm = np.ascontiguousarray(b[rv].reshape(1, EV))
        in_maps.append(
            {"xT": xTs[n], "wkq": wkqm, "wv": wvm, "bkq": bkqm, "bv": bvm}
        )
    return in_maps


def run(inputs, **kwargs):
    nc = _get_nc()
    in_maps = make_in_maps(inputs["x"], inputs["W"], inputs["b"])
    res = run_bass_kernel_spmd(nc, in_maps, core_ids=list(range(8)), **kwargs)
    out = np.empty((N, S, D), dtype=np.float32)
    for c in range(8):
        n, g = divmod(c, 4)
        out[n, :, 256 * g:256 * g + 256] = res.results[c]["o"]
    return out, res


def kernel(**inputs):
    out, _ = run(inputs)
    return out
